# revision 1
# baseline (speedup 1.0000x reference)
"""Trainium2 Bass kernel for the grouped TF->gene sparse decoder (AEDecoder).

Math (reference):
  h1 = leaky_relu(features[:,:,None] * w1 + b1)            # [B,T,K]
  h2 = leaky_relu(einsum('btj,tjk->btk', h1, w2) + b2)     # [B,T,K]
  out = einsum('bgek,gek->bg', h2[:, edge_tf, :], w3) + b3 # [B,G]

Sparse run-length formulation:
  The final contraction touches only 12 of the 2048 (t,k) rows per gene
  (3 edges x K).  Rows fall in 16 chunks of 128 partitions; a gene touches
  <=3 distinct chunks (avg 2.82).  Genes are sorted globally by their
  (c1<=c2<=c3) chunk triple and dealt round-robin to the 8 cores, so all
  cores share ONE instruction template (run boundaries agree within +-1
  column across cores; padded to the max with zero S-columns) while the S
  data differs per core.  The host un-permutes the gene order at gather.

  Per batch-tile (128 cells), chunks run in ascending order: a gene's
  first chunk writes psum with start=True (level-1 runs contiguous by
  construction; one ambiguous boundary column per block pair gets a
  start@c1 + accum@c1' 1-col pair), later chunks accumulate (level-2/3
  runs contiguous within parent blocks).  Streamed cols ~7.6k/btile vs
  41k dense.  LDWEIGHTS elision (walrus --enable-ldw-opt) keeps the ~45
  same-stationary matmuls per chunk cheap.

  h1 on DVE (tensor_scalar affine + scalar_tensor_tensor leaky); h2 = ACT
  Prelu over the PE block-diag w2 matmul (psum ping-pong banks 5,6).  b3
  is added by a contraction-1 matmul (ones x b3row) closing each psum
  bank; evictions (psum -> bf16 SBUF) alternate ACT/DVE; per-bank out
  DMA.  The 8 psum banks rotate through 4 btiles x 5 bank-slots; btile1's
  bank-7 slot runs early (interleaved into btile0) to hide h2-build
  pacing.  Spack/w2blk stream on the gpsimd DMA queue, featT/cols/b3 on
  the sync queue.

Sharding: 8 cores x 2500 genes (dealt), full batch per core; out bf16
[512, 2500] per core, host casts to fp32 and un-permutes.
"""

import os

import numpy as np
import ml_dtypes

import concourse.bass as bass
import concourse.mybir as mybir
from concourse.bass_utils import run_bass_kernel_spmd

BF16 = mybir.dt.bfloat16
F32 = mybir.dt.float32
AFT = mybir.ActivationFunctionType
ALU = mybir.AluOpType

B, T, K, G, EPG = 512, 512, 4, 20000, 3
NCORES = 8
GSH = G // NCORES            # 2500 genes per core
NCH = (T * K) // 128         # 16 contract chunks (h-build granularity)
NSC = 8                      # 8 superchunks of 256 rows for the main matmul
SUBS = 2                     # partition chunks per superchunk
NBT = B // 128               # 4 batch tiles
NSLOT = (GSH + 511) // 512   # 5 psum bank-slots per btile
ALPHA = 0.01

# (btile, slot) -> psum bank ring; b3/eviction order = PE completion order
BANK = lambda m, j: (5 * m + j) % 8
EV_LIST = ([(0, j) for j in range(5)] + [(1, 2), (1, 0), (1, 1), (1, 3), (1, 4)]
           + [(2, j) for j in range(5)] + [(3, j) for j in range(5)])
EV_RANK = {mj: e for e, mj in enumerate(EV_LIST)}

_CACHE = {}
LAST_RESULT = None
_LDW_PATCHED = False


def _enable_ldw_opt():
    """Flip walrus --enable-ldw-opt to true: elides redundant LDWEIGHTS for
    back-to-back matmuls sharing a stationary operand (our per-chunk run
    lists reuse one h2 block across ~45 matmuls)."""
    global _LDW_PATCHED
    if _LDW_PATCHED or not os.environ.get("KERNEL_LDW_OPT"):
        return
    import concourse.bass_utils as bu
    orig = bu.run_command

    def _run(cmd, **kw):
        new = ["--enable-ldw-opt=true" if c == "--enable-ldw-opt=false" else c
               for c in cmd]
        if new != cmd and os.environ.get("KERNEL_DEBUG"):
            print("[ldw-opt] flag flipped in walrus cmd")
        return orig(new, **kw)

    bu.run_command = _run
    _LDW_PATCHED = True


def _ensure_profile_hook():
    """Register an NTFF profile hook when the image lacks antenv.axon_hooks."""
    import contextlib
    import ctypes
    import sys
    import types

    try:
        import antenv.axon_hooks  # noqa: F401
        return
    except ImportError:
        pass

    holder = {}
    mod = types.ModuleType("antenv.axon_hooks")
    mod.set_axon_ntff_profile_hook = lambda h: holder.__setitem__("h", h)
    mod.get_axon_ntff_profile_hook = lambda: holder.get("h")
    sys.modules["antenv.axon_hooks"] = mod

    so_path = "/opt/axon/libaxon_pjrt.so"
    try:
        lib = ctypes.CDLL(so_path)
    except OSError:
        return
    if not hasattr(lib, "axon_start_nrt_profile"):
        return
    lib.axon_start_nrt_profile.argtypes = [
        ctypes.POINTER(ctypes.c_int64), ctypes.c_size_t,
    ]
    lib.axon_start_nrt_profile.restype = ctypes.c_int64
    lib.axon_stop_nrt_profile.argtypes = [ctypes.c_char_p]
    lib.axon_stop_nrt_profile.restype = ctypes.c_int64

    @contextlib.contextmanager
    def _hook(output_dir, device_ids):
        import jax
        jax.devices()
        if device_ids:
            ids = (ctypes.c_int64 * len(device_ids))(*device_ids)
            rc = lib.axon_start_nrt_profile(ids, len(device_ids))
        else:
            rc = lib.axon_start_nrt_profile(None, 0)
        if rc != 0:
            raise RuntimeError(f"axon_start_nrt_profile rc={rc}")
        try:
            yield
        finally:
            n = lib.axon_stop_nrt_profile(str(output_dir).encode())
            print(f"profile: {n} ntff file(s) written to {output_dir}")

    holder["h"] = _hook

    import concourse.bass_utils as bu
    bu.upload_artifacts = lambda tmpdir: tmpdir


# ---------------------------------------------------------------------------
# Template: global gene sort + round-robin deal -> per-chunk piece lists
# shared by all 8 cores.  Pure function of edge_tf.
# ---------------------------------------------------------------------------

def _build_template(edge_tf):
    chunk = edge_tf // 64                      # [G, EPG] superchunk (256 rows)
    keys = np.full((G, 3), NSC, np.int64)      # sorted distinct, pad NSC
    for g in range(G):
        cs = sorted(set(chunk[g].tolist()))
        keys[g, : len(cs)] = cs
    order = np.lexsort((keys[:, 2], keys[:, 1], keys[:, 0]))
    sk = keys[order]

    def blocks(ncols):
        a = sk[:, :ncols]
        change = np.any(a[1:] != a[:-1], axis=1)
        bounds = [0] + (np.nonzero(change)[0] + 1).tolist() + [len(a)]
        for i in range(len(bounds) - 1):
            yield tuple(a[bounds[i]].tolist()), bounds[i], bounds[i + 1]

    # runs: (sc, kind, lo, hi, blockkey, level); positions in [0, GSH)
    runs = []
    l1 = list(blocks(1))
    for i, ((c1,), A, Bb) in enumerate(l1):
        lo, hi = (A + 7) // 8, Bb // 8
        if hi > lo:
            runs.append((c1, "start", lo, hi, (c1,), 1))
        if Bb % 8 != 0 and Bb < G:
            c1n = l1[i + 1][0][0]
            runs.append((c1, "amb_s", Bb // 8, Bb // 8 + 1, (c1,), 1))
            runs.append((c1n, "amb_a", Bb // 8, Bb // 8 + 1, (c1n,), 1))
    for (c1, c2), A, Bb in blocks(2):
        if c2 == NSC:
            continue
        runs.append((c2, "accum", A // 8, (Bb + 7) // 8, (c1, c2), 2))
    for (c1, c2, c3), A, Bb in blocks(3):
        if c3 == NSC:
            continue
        runs.append((c3, "accum", A // 8, (Bb + 7) // 8, (c1, c2, c3), 3))

    # emission order: by superchunk ascending; within one, starts first
    kindord = {"start": 0, "amb_s": 1, "amb_a": 2, "accum": 3}
    runs.sort(key=lambda r: (r[0], kindord[r[1]], r[2]))

    # Each run expands to SUBS matmuls (contraction 256 = 2 partition chunks);
    # spack stores the run's sub-0 block then sub-1 block.  Pieces split at
    # psum bank (512-col) boundaries.
    # HW: start=True resets the ENTIRE psum bank, so exactly one matmul per
    # bank-slot (the first in emission order) carries start=True; everything
    # else accumulates onto the zeroed bank.
    pieces = []          # (sc, psum_lo, psum_hi, spack_lo_run, run_lo, width)
    run_off = []         # spack offset of each run (sub-0 block)
    off = 0
    for c, kind, lo, hi, bk, lvl in runs:
        run_off.append(off)
        p = lo
        while p < hi:
            q = min(hi, (p // 512 + 1) * 512)
            pieces.append((c, p, q, off, lo, hi - lo))
            p = q
        off += SUBS * (hi - lo)
    ncols = off

    # sc_pieces[S] = [(is_start, sub, plo, phi, slo), ...] emission order:
    # sub-major within a superchunk so same-stationary matmuls are adjacent
    sc_pieces = {c: [] for c in range(NSC)}
    tmp = {c: [] for c in range(NSC)}
    for c, plo, phi, off0, rlo, rw in pieces:
        tmp[c].append((plo, phi, off0, rlo, rw))
    slot_seen = set()
    for c in range(NSC):
        for sub in range(SUBS):
            for plo, phi, off0, rlo, rw in tmp[c]:
                slo = off0 + sub * rw + (plo - rlo)
                j = plo // 512
                is_start = j not in slot_seen
                slot_seen.add(j)
                sc_pieces[c].append((is_start, sub, plo, phi, slo))
    # spack DMA groups: one per superchunk
    grp_hi = []
    for jc in range(NSC):
        nxt = [run_off[i] for i, r in enumerate(runs) if r[0] > jc]
        grp_hi.append(min(nxt) if nxt else ncols)

    return dict(keys=keys, order=order, runs=runs, run_off=run_off,
                ncols=ncols, sc_pieces=sc_pieces, grp_hi=grp_hi,
                chunkmap=chunk)


# ---------------------------------------------------------------------------
# Host data packing (layout/index preprocessing only)
# ---------------------------------------------------------------------------

def _prep_inputs(tpl, features, w1, b1, w2, b2, w3, b3, edge_tf):
    bf = ml_dtypes.bfloat16
    keys, order, runs = tpl["keys"], tpl["order"], tpl["runs"]
    run_off, ncols = tpl["run_off"], tpl["ncols"]

    featT = np.repeat(np.ascontiguousarray(features.T), K, axis=0)
    featT = np.ascontiguousarray(
        featT.reshape(NCH, 128, B).transpose(1, 0, 2)).astype(bf)

    w1c = w1.reshape(T * K).reshape(NCH, 128).T.astype(np.float32)
    b1c = b1.reshape(T * K).reshape(NCH, 128).T.astype(np.float32)
    b2c = b2.reshape(T * K).reshape(NCH, 128).T.astype(np.float32)
    cols = np.concatenate([w1c, b1c, b2c], axis=1).copy()

    w2r = w2.reshape(NCH, 32, K, K)
    w2blk = np.zeros((NCH, 32, K, 32, K), np.float32)
    for i in range(32):
        w2blk[:, i, :, i, :] = w2r[:, i]
    w2blk = np.ascontiguousarray(
        w2blk.reshape(NCH, 128, 128).transpose(1, 0, 2)).astype(bf)

    # per-gene merged columns per distinct superchunk slot, per sub-chunk
    gcol = np.zeros((G, 3, SUBS, 128), np.float32)
    gidx = np.arange(G)
    for e in range(EPG):
        t = edge_tf[:, e]
        cc = t // 64
        s = np.argmax(keys == cc[:, None], axis=1)
        sub = (t % 64) // 32
        rows = 4 * (t % 32)
        for k in range(K):
            np.add.at(gcol, (gidx, s, sub, rows + k), w3[:, e, k])

    gcore = np.empty((NCORES, GSH), np.int64)      # position -> original gene
    for core in range(NCORES):
        gcore[core] = order[np.arange(GSH) * 8 + core]

    spack = np.zeros((NCORES, 128, ncols), np.float32)
    for ri, (c, kind, lo, hi, bk, lvl) in enumerate(runs):
        w = hi - lo
        o = run_off[ri]
        ps = np.arange(lo, hi)
        for core in range(NCORES):
            genes = gcore[core][ps]
            kk = keys[genes]
            member = kk[:, 0] == bk[0]
            for d in range(1, lvl):
                member &= kk[:, d] == bk[d]
            s = np.argmax(kk == c, axis=1)
            for sub in range(SUBS):
                vals = np.where(member[:, None], gcol[genes, s, sub, :], 0.0)
                spack[core, :, o + sub * w : o + (sub + 1) * w] = vals.T
    spack = spack.astype(bf)

    b3p = np.zeros((NCORES, 1, GSH), np.float32)
    for core in range(NCORES):
        b3p[core, 0, :] = b3[gcore[core]]
    b3p = b3p.astype(bf)

    in_maps = []
    for core in range(NCORES):
        in_maps.append({
            "featT": featT,
            "cols": cols,
            "W2blk": w2blk,
            "Spack": np.ascontiguousarray(spack[core]),
            "B3p": np.ascontiguousarray(b3p[core]),
        })
    return in_maps, gcore


# ---------------------------------------------------------------------------
# Graph
# ---------------------------------------------------------------------------

def _build_graph(tpl):
    from contextlib import ExitStack

    ncols = tpl["ncols"]
    sc_pieces = tpl["sc_pieces"]
    grp_hi = tpl["grp_hi"]

    nc = bass.Bass()
    featT_h = nc.declare_dram_parameter("featT", [128, NCH, B], BF16, isOutput=False)
    cols_h = nc.declare_dram_parameter("cols", [128, 3 * NCH], F32, isOutput=False)
    w2blk_h = nc.declare_dram_parameter("W2blk", [128, NCH, 128], BF16, isOutput=False)
    spack_h = nc.declare_dram_parameter("Spack", [128, ncols], BF16, isOutput=False)
    b3p_h = nc.declare_dram_parameter("B3p", [1, GSH], BF16, isOutput=False)
    out_h = nc.declare_dram_parameter("out", [B, GSH], BF16, isOutput=True)

    def slot_w(j):
        return min(GSH - 512 * j, 512)

    with ExitStack() as es:
        featT = es.enter_context(nc.sbuf_tensor("ft_sb", [128, NCH, B], BF16))
        colsb = es.enter_context(nc.sbuf_tensor("cols_sb", [128, 3 * NCH], F32))
        w2blk = es.enter_context(nc.sbuf_tensor("w2_sb", [128, NCH, 128], BF16))
        spk = es.enter_context(nc.sbuf_tensor("spk_sb", [128, ncols], BF16))
        b3sb = es.enter_context(nc.sbuf_tensor("b3_sb", [1, GSH], BF16))
        ones = es.enter_context(nc.sbuf_tensor("ones_sb", [1, 128], BF16))
        tbuf = es.enter_context(nc.sbuf_tensor("t_sb", [128, B], BF16))
        h1 = es.enter_context(nc.sbuf_tensor("h1_sb", [128, NCH, B], BF16))
        h2 = es.enter_context(nc.sbuf_tensor("h2_sb", [128, NCH, B], BF16))
        outsb = es.enter_context(nc.sbuf_tensor("out_sb", [128, NBT, 512 * NSLOT], BF16))
        touch = es.enter_context(nc.sbuf_tensor("touch_sb", [128, 4], BF16))
        pm = [es.enter_context(nc.psum_tensor(f"pm{j}", [128, 512], F32))
              for j in range(8)]

        w1a = colsb[:, 0:NCH]
        b1a = colsb[:, NCH : 2 * NCH]
        b2a = colsb[:, 2 * NCH : 3 * NCH]

        # DMA chain order on the single sync queue (baseline-proven sems):
        # cols, b3, w2blk, fq0, sp0, fq1, sp1, fq2, sp2, fq3, sp3..sp7
        FQ_INC = [16 * p for p in (4, 6, 8, 10)]          # featT quarter pos
        SP_INC = [16 * p for p in (5, 7, 9, 11, 12, 13, 14, 15)]

        with (
            nc.Block() as block,
            nc.semaphore("dsync") as dsync,    # single DMA chain
            nc.semaphore("peh") as sem_peh,    # PE w2-mm per chunk
            nc.semaphore("act") as sem_act,    # ACT h1/h2, 2 per chunk
            nc.semaphore("pem") as sem_pem,    # PE bank complete (b3-mm)
            nc.semaphore("ev") as sem_ev,      # DVE evictions (ordered)
            nc.semaphore("od") as sem_od,      # out DMA
        ):
            def ev_wait(engine, m, j):
                """Wait for the previous tenant of bank BANK(m,j) to evict."""
                prev = {(1, 3): (0, 0), (1, 4): (0, 1), (2, 0): (0, 2),
                        (2, 1): (0, 3), (2, 2): (0, 4), (2, 3): (1, 0),
                        (2, 4): (1, 1), (3, 0): (1, 2), (3, 1): (1, 3),
                        (3, 2): (1, 4), (3, 3): (2, 0), (3, 4): (2, 1)}.get((m, j))
                if prev is not None:
                    engine.wait_ge(sem_ev, EV_RANK[prev] + 1)

            @block.sync
            def _(sync: bass.BassEngine):
                sync.dma_start(out=colsb[:], in_=cols_h[:]).then_inc(dsync, 16)
                sync.dma_start(out=b3sb[:], in_=b3p_h[:]).then_inc(dsync, 16)
                sync.dma_start(out=w2blk[:], in_=w2blk_h[:]).then_inc(dsync, 16)
                sp_bounds = [0] + list(grp_hi)
                for q in range(4):
                    sync.dma_start(
                        out=featT[:, 4 * q : 4 * (q + 1), :],
                        in_=featT_h[:, 4 * q : 4 * (q + 1), :],
                    ).then_inc(dsync, 16)
                    lo, hi = sp_bounds[q], sp_bounds[q + 1]
                    sync.dma_start(
                        out=spk[:, lo : max(hi, lo + 1)],
                        in_=spack_h[:, lo : max(hi, lo + 1)],
                    ).then_inc(dsync, 16)
                for q in range(4, 8):
                    lo, hi = sp_bounds[q], sp_bounds[q + 1]
                    sync.dma_start(
                        out=spk[:, lo : max(hi, lo + 1)],
                        in_=spack_h[:, lo : max(hi, lo + 1)],
                    ).then_inc(dsync, 16)
                for e, (m, j) in enumerate(EV_LIST):
                    sync.wait_ge(sem_ev, e + 1)
                    w = slot_w(j)
                    sync.dma_start(
                        out=out_h[m * 128 : (m + 1) * 128, 512 * j : 512 * j + w],
                        in_=outsb[:, m, 512 * j : 512 * j + w],
                    ).then_inc(sem_od, 16)
                sync.wait_ge(sem_od, 16 * len(EV_LIST))

            @block.vector
            def _(vector: bass.BassEngine):
                vector.memset(ones[:], 1.0)
                for e, (m, j) in enumerate(EV_LIST):
                    w = slot_w(j)
                    vector.wait_ge(sem_pem, e + 1)
                    vector.tensor_scalar_add(
                        outsb[:, m, 512 * j : 512 * j + w],
                        pm[BANK(m, j)][:, :w], 0.0,
                    ).then_inc(sem_ev)

            @block.scalar
            def _(scalar: bass.BassEngine):
                # baseline-proven producer: ACT does both h1 (from SBUF) and
                # h2 (from the W2 psum), incrementing sem_act twice per chunk
                for c in range(NCH):
                    scalar.wait_ge(dsync, FQ_INC[c // 4])
                    scalar.activation(
                        h1[:, c, :], featT[:, c, :], AFT.Prelu,
                        bias=b1a[:, c : c + 1], scale=w1a[:, c : c + 1],
                        alpha=ALPHA,
                    ).then_inc(sem_act)
                    scalar.wait_ge(sem_peh, c + 1)
                    scalar.activation(
                        h2[:, c, :], pm[5 + c % 2][:, :], AFT.Prelu,
                        bias=b2a[:, c : c + 1], alpha=ALPHA,
                    ).then_inc(sem_act)

            @block.tensor
            def _(tensor: bass.BassEngine):
                def warm(k, n=512):
                    for _ in range(k):
                        tensor.matmul(
                            pm[7][:, :n], featT[:, 0, 0:128], featT[:, 0, :n],
                            start=True, stop=True, skip_group_check=True,
                        )

                def emit_runs(m, sc, slots):
                    for is_start, sub, plo, phi, slo in sc_pieces[sc]:
                        j = plo // 512
                        if j not in slots:
                            continue
                        w = phi - plo
                        tensor.matmul(
                            pm[BANK(m, j)][:, plo - 512 * j : phi - 512 * j],
                            h2[:, SUBS * sc + sub, m * 128 : (m + 1) * 128],
                            spk[:, slo : slo + w],
                            start=is_start, stop=False, skip_group_check=True,
                        )

                def b3mm(m, j):
                    w = slot_w(j)
                    tensor.matmul(
                        pm[BANK(m, j)][:, :w], ones[0:1, 0:128],
                        b3sb[0:1, 512 * j : 512 * j + w],
                        start=False, stop=True, skip_group_check=True,
                    ).then_inc(sem_pem)

                def w2mm(c):
                    if c == 0:
                        tensor.wait_ge(dsync, 48)       # w2blk
                    tensor.wait_ge(sem_act, 2 * c + 1)  # h1(c) written
                    tensor.matmul(
                        pm[5 + c % 2][:, :], w2blk[:, c, :], h1[:, c, :],
                        start=True, stop=True,
                    ).then_inc(sem_peh)

                warm(5)
                # build + btile0 (+ btile1's bank-7 slot j=2)
                for sc in range(NSC):
                    w2mm(2 * sc)
                    w2mm(2 * sc + 1)
                    tensor.wait_ge(sem_act, 2 * (2 * sc + 1) + 2)  # h2 ready
                    tensor.wait_ge(dsync, SP_INC[sc])   # spack group
                    emit_runs(0, sc, (0, 1, 2, 3, 4))
                    emit_runs(1, sc, (2,))
                tensor.wait_ge(dsync, 32)               # b3sb
                for j in range(5):
                    b3mm(0, j)
                b3mm(1, 2)
                # btile1 slots 0,1 (banks 5,6 -- free once ACT consumed ph)
                for sc in range(NSC):
                    emit_runs(1, sc, (0, 1))
                b3mm(1, 0)
                b3mm(1, 1)
                # btile1 slots 3,4 (banks 0,1 <- evictions of t0 j0,j1)
                ev_wait(tensor, 1, 3)
                ev_wait(tensor, 1, 4)
                for sc in range(NSC):
                    emit_runs(1, sc, (3, 4))
                b3mm(1, 3)
                b3mm(1, 4)
                # btile2
                for j in range(5):
                    ev_wait(tensor, 2, j)
                for sc in range(NSC):
                    emit_runs(2, sc, (0, 1, 2, 3, 4))
                for j in range(5):
                    b3mm(2, j)
                # btile3
                for j in range(5):
                    ev_wait(tensor, 3, j)
                for sc in range(NSC):
                    emit_runs(3, sc, (0, 1, 2, 3, 4))
                for j in range(5):
                    b3mm(3, j)

    return nc


def kernel(features, w1, b1, w2, b2, w3, b3, edge_tf):
    global LAST_RESULT
    features, w1, b1, w2, b2, w3, b3, edge_tf = (
        np.asarray(x) for x in (features, w1, b1, w2, b2, w3, b3, edge_tf)
    )
    key = hash(edge_tf.tobytes())
    if key not in _CACHE:
        tpl = _build_template(edge_tf)
        _CACHE.clear()
        _CACHE[key] = (tpl, _build_graph(tpl))
    tpl, graph = _CACHE[key]

    in_maps, gcore = _prep_inputs(
        tpl, features, w1, b1, w2, b2, w3, b3, edge_tf)
    trace = bool(int(os.environ.get("KERNEL_TRACE", "0")))
    if trace:
        _ensure_profile_hook()
    _enable_ldw_opt()
    res = run_bass_kernel_spmd(
        graph, in_maps, core_ids=list(range(NCORES)), trace=trace,
    )
    LAST_RESULT = res
    out = np.zeros((B, G), np.float32)
    for core in range(NCORES):
        dev = np.asarray(res.results[core]["out"]).astype(np.float32)
        out[:, gcore[core]] = dev
    return out



# revision 2
# speedup vs baseline: 1.1376x; 1.1376x over previous
"""Trainium2 Bass kernel for the grouped TF->gene sparse decoder (AEDecoder).

Math (reference):
  h1 = leaky_relu(features[:,:,None] * w1 + b1)            # [B,T,K]
  h2 = leaky_relu(einsum('btj,tjk->btk', h1, w2) + b2)     # [B,T,K]
  out = einsum('bgek,gek->bg', h2[:, edge_tf, :], w3) + b3 # [B,G]

Sparse run-length formulation:
  The final contraction touches only 12 of the 2048 (t,k) rows per gene
  (3 edges x K).  Rows fall in 8 superchunks of 256; a gene touches
  <=3 distinct superchunks (avg 2.64).  Genes are sorted globally by their
  (c1<=c2<=c3) triple and dealt round-robin to the 8 cores, so all
  cores share ONE instruction template while the S data differs per core.
  The host un-permutes the gene order at gather.

Schedule (v2 — HAM-warm + parallel DMA + split h-build):
  * The PE HAM clock gate defaults to 1.2 GHz and only reaches 2.4 GHz
    after ~3.4us of sustained matmul activity.  Warmup matmuls run from
    block entry through the DMA-wait window so the real stream starts and
    stays warm (no PE gaps > ~1us mid-kernel).
  * Input DMA is split across both HWDGE rings: featT+cols issue from the
    scalar (ACT) engine queue, w2blk+b3+spack from the sync queue.  This
    parallelizes the ~0.7us/descriptor issue cost and lands featT ~6us
    earlier.
  * h-build is pipelined across three engines: DVE computes h1
    (tensor_scalar affine + scalar_tensor_tensor leaky max(x, 0.01x)),
    PE does the block-diag w2 matmul (psum ping-pong banks 5,6), ACT does
    h2 = Prelu(psum + b2).  Per-chunk critical path drops ~2.6us -> ~0.75us.
  * Main stream: per batch-tile, runs accumulate into psum banks in
    ascending superchunk order (one start=True per bank); b3 is added by a
    contraction-1 matmul closing each bank; DVE evicts psum -> bf16 SBUF;
    per-bank out DMA on the sync queue.  The 8 psum banks rotate through
    4 btiles x 5 bank-slots; btile1's bank-7 slot runs during the build.

Sharding: 8 cores x 2500 genes (dealt), full batch per core; out bf16
[512, 2500] per core, host casts to fp32 and un-permutes.
"""

import os

import numpy as np
import ml_dtypes

import concourse.bass as bass
import concourse.mybir as mybir
from concourse.bass_utils import run_bass_kernel_spmd

BF16 = mybir.dt.bfloat16
F32 = mybir.dt.float32
AFT = mybir.ActivationFunctionType
ALU = mybir.AluOpType

B, T, K, G, EPG = 512, 512, 4, 20000, 3
NCORES = 8
GSH = G // NCORES            # 2500 genes per core
NCH = (T * K) // 128         # 16 contract chunks (h-build granularity)
NSC = 8                      # 8 superchunks of 256 rows for the main matmul
SUBS = 2                     # partition chunks per superchunk
NBT = B // 128               # 4 batch tiles
NSLOT = (GSH + 511) // 512   # 5 psum bank-slots per btile
ALPHA = 0.01
NWARM = 9                    # HAM warmup matmuls (512 cols, cold ~0.43us each)

# (btile, slot) -> psum bank ring; b3/eviction order = PE completion order
BANK = lambda m, j: (5 * m + j) % 8
EV_LIST = ([(0, j) for j in range(5)] + [(1, 2), (1, 0), (1, 1), (1, 3), (1, 4)]
           + [(2, j) for j in range(5)] + [(3, j) for j in range(5)])
EV_RANK = {mj: e for e, mj in enumerate(EV_LIST)}

_CACHE = {}
LAST_RESULT = None
_LDW_PATCHED = False


def _enable_ldw_opt():
    """Flip walrus --enable-ldw-opt to true: elides redundant LDWEIGHTS for
    back-to-back matmuls sharing a stationary operand (our per-chunk run
    lists reuse one h2 block across ~45 matmuls)."""
    global _LDW_PATCHED
    if _LDW_PATCHED:
        return
    import concourse.bass_utils as bu
    orig = bu.run_command

    def _run(cmd, **kw):
        new = ["--enable-ldw-opt=true" if c == "--enable-ldw-opt=false" else c
               for c in cmd]
        return orig(new, **kw)

    bu.run_command = _run
    _LDW_PATCHED = True


def _ensure_profile_hook():
    """Register an NTFF profile hook when the image lacks antenv.axon_hooks."""
    import contextlib
    import ctypes
    import sys
    import types

    try:
        import antenv.axon_hooks  # noqa: F401
        return
    except ImportError:
        pass

    holder = {}
    mod = types.ModuleType("antenv.axon_hooks")
    mod.set_axon_ntff_profile_hook = lambda h: holder.__setitem__("h", h)
    mod.get_axon_ntff_profile_hook = lambda: holder.get("h")
    sys.modules["antenv.axon_hooks"] = mod

    so_path = "/opt/axon/libaxon_pjrt.so"
    try:
        lib = ctypes.CDLL(so_path)
    except OSError:
        return
    if not hasattr(lib, "axon_start_nrt_profile"):
        return
    lib.axon_start_nrt_profile.argtypes = [
        ctypes.POINTER(ctypes.c_int64), ctypes.c_size_t,
    ]
    lib.axon_start_nrt_profile.restype = ctypes.c_int64
    lib.axon_stop_nrt_profile.argtypes = [ctypes.c_char_p]
    lib.axon_stop_nrt_profile.restype = ctypes.c_int64

    @contextlib.contextmanager
    def _hook(output_dir, device_ids):
        import jax
        jax.devices()
        if device_ids:
            ids = (ctypes.c_int64 * len(device_ids))(*device_ids)
            rc = lib.axon_start_nrt_profile(ids, len(device_ids))
        else:
            rc = lib.axon_start_nrt_profile(None, 0)
        if rc != 0:
            raise RuntimeError(f"axon_start_nrt_profile rc={rc}")
        try:
            yield
        finally:
            n = lib.axon_stop_nrt_profile(str(output_dir).encode())
            print(f"profile: {n} ntff file(s) written to {output_dir}")

    holder["h"] = _hook

    import concourse.bass_utils as bu
    bu.upload_artifacts = lambda tmpdir: tmpdir


# ---------------------------------------------------------------------------
# Template: global gene sort + round-robin deal -> per-chunk piece lists
# shared by all 8 cores.  Pure function of edge_tf.
# ---------------------------------------------------------------------------

def _build_template(edge_tf):
    chunk = edge_tf // 64                      # [G, EPG] superchunk (256 rows)
    keys = np.full((G, 3), NSC, np.int64)      # sorted distinct, pad NSC
    for g in range(G):
        cs = sorted(set(chunk[g].tolist()))
        keys[g, : len(cs)] = cs
    order = np.lexsort((keys[:, 2], keys[:, 1], keys[:, 0]))
    sk = keys[order]

    def blocks(ncols):
        a = sk[:, :ncols]
        change = np.any(a[1:] != a[:-1], axis=1)
        bounds = [0] + (np.nonzero(change)[0] + 1).tolist() + [len(a)]
        for i in range(len(bounds) - 1):
            yield tuple(a[bounds[i]].tolist()), bounds[i], bounds[i + 1]

    # runs: (sc, kind, lo, hi, blockkey, level); positions in [0, GSH)
    runs = []
    l1 = list(blocks(1))
    for i, ((c1,), A, Bb) in enumerate(l1):
        lo, hi = (A + 7) // 8, Bb // 8
        if hi > lo:
            runs.append((c1, "start", lo, hi, (c1,), 1))
        if Bb % 8 != 0 and Bb < G:
            c1n = l1[i + 1][0][0]
            runs.append((c1, "amb_s", Bb // 8, Bb // 8 + 1, (c1,), 1))
            runs.append((c1n, "amb_a", Bb // 8, Bb // 8 + 1, (c1n,), 1))
    for (c1, c2), A, Bb in blocks(2):
        if c2 == NSC:
            continue
        runs.append((c2, "accum", A // 8, (Bb + 7) // 8, (c1, c2), 2))
    for (c1, c2, c3), A, Bb in blocks(3):
        if c3 == NSC:
            continue
        runs.append((c3, "accum", A // 8, (Bb + 7) // 8, (c1, c2, c3), 3))

    # emission order: by superchunk ascending; within one, starts first
    kindord = {"start": 0, "amb_s": 1, "amb_a": 2, "accum": 3}
    runs.sort(key=lambda r: (r[0], kindord[r[1]], r[2]))

    # Each run expands to SUBS matmuls (contraction 256 = 2 partition chunks);
    # spack stores the run's sub-0 block then sub-1 block.  Pieces split at
    # psum bank (512-col) boundaries.
    # HW: start=True resets the ENTIRE psum bank, so exactly one matmul per
    # bank-slot (the first in emission order) carries start=True; everything
    # else accumulates onto the zeroed bank.
    pieces = []          # (sc, psum_lo, psum_hi, spack_lo_run, run_lo, width)
    run_off = []         # spack offset of each run (sub-0 block)
    off = 0
    for c, kind, lo, hi, bk, lvl in runs:
        run_off.append(off)
        p = lo
        while p < hi:
            q = min(hi, (p // 512 + 1) * 512)
            pieces.append((c, p, q, off, lo, hi - lo))
            p = q
        off += SUBS * (hi - lo)
    ncols = off

    # sc_pieces[S] = [(is_start, sub, plo, phi, slo), ...] emission order:
    # sub-major within a superchunk so same-stationary matmuls are adjacent
    sc_pieces = {c: [] for c in range(NSC)}
    tmp = {c: [] for c in range(NSC)}
    for c, plo, phi, off0, rlo, rw in pieces:
        tmp[c].append((plo, phi, off0, rlo, rw))
    slot_seen = set()
    for c in range(NSC):
        for sub in range(SUBS):
            for plo, phi, off0, rlo, rw in tmp[c]:
                slo = off0 + sub * rw + (plo - rlo)
                j = plo // 512
                is_start = j not in slot_seen
                slot_seen.add(j)
                sc_pieces[c].append((is_start, sub, plo, phi, slo))
    # spack DMA groups: one per superchunk
    grp_hi = []
    for jc in range(NSC):
        nxt = [run_off[i] for i, r in enumerate(runs) if r[0] > jc]
        grp_hi.append(min(nxt) if nxt else ncols)

    return dict(keys=keys, order=order, runs=runs, run_off=run_off,
                ncols=ncols, sc_pieces=sc_pieces, grp_hi=grp_hi,
                chunkmap=chunk)


# ---------------------------------------------------------------------------
# Host data packing (layout/index preprocessing only)
# ---------------------------------------------------------------------------

def _prep_inputs(tpl, features, w1, b1, w2, b2, w3, b3, edge_tf):
    bf = ml_dtypes.bfloat16
    keys, order, runs = tpl["keys"], tpl["order"], tpl["runs"]
    run_off, ncols = tpl["run_off"], tpl["ncols"]

    featT = np.repeat(np.ascontiguousarray(features.T), K, axis=0)
    featT = np.ascontiguousarray(
        featT.reshape(NCH, 128, B).transpose(1, 0, 2)).astype(bf)

    w1c = w1.reshape(T * K).reshape(NCH, 128).T.astype(np.float32)
    b1c = b1.reshape(T * K).reshape(NCH, 128).T.astype(np.float32)
    b2c = b2.reshape(T * K).reshape(NCH, 128).T.astype(np.float32)
    cols = np.concatenate([w1c, b1c, b2c], axis=1).copy()

    w2r = w2.reshape(NCH, 32, K, K)
    w2blk = np.zeros((NCH, 32, K, 32, K), np.float32)
    for i in range(32):
        w2blk[:, i, :, i, :] = w2r[:, i]
    w2blk = np.ascontiguousarray(
        w2blk.reshape(NCH, 128, 128).transpose(1, 0, 2)).astype(bf)

    # per-gene merged columns per distinct superchunk slot, per sub-chunk
    gcol = np.zeros((G, 3, SUBS, 128), np.float32)
    gidx = np.arange(G)
    for e in range(EPG):
        t = edge_tf[:, e]
        cc = t // 64
        s = np.argmax(keys == cc[:, None], axis=1)
        sub = (t % 64) // 32
        rows = 4 * (t % 32)
        for k in range(K):
            np.add.at(gcol, (gidx, s, sub, rows + k), w3[:, e, k])

    gcore = np.empty((NCORES, GSH), np.int64)      # position -> original gene
    for core in range(NCORES):
        gcore[core] = order[np.arange(GSH) * 8 + core]

    spack = np.zeros((NCORES, 128, ncols), np.float32)
    for ri, (c, kind, lo, hi, bk, lvl) in enumerate(runs):
        w = hi - lo
        o = run_off[ri]
        ps = np.arange(lo, hi)
        for core in range(NCORES):
            genes = gcore[core][ps]
            kk = keys[genes]
            member = kk[:, 0] == bk[0]
            for d in range(1, lvl):
                member &= kk[:, d] == bk[d]
            s = np.argmax(kk == c, axis=1)
            for sub in range(SUBS):
                vals = np.where(member[:, None], gcol[genes, s, sub, :], 0.0)
                spack[core, :, o + sub * w : o + (sub + 1) * w] = vals.T
    spack = spack.astype(bf)

    b3p = np.zeros((NCORES, 1, GSH), np.float32)
    for core in range(NCORES):
        b3p[core, 0, :] = b3[gcore[core]]
    b3p = b3p.astype(bf)

    in_maps = []
    for core in range(NCORES):
        in_maps.append({
            "featT": featT,
            "cols": cols,
            "W2blk": w2blk,
            "Spack": np.ascontiguousarray(spack[core]),
            "B3p": np.ascontiguousarray(b3p[core]),
        })
    return in_maps, gcore


# ---------------------------------------------------------------------------
# Graph
# ---------------------------------------------------------------------------

def _build_graph(tpl):
    from contextlib import ExitStack

    ncols = tpl["ncols"]
    sc_pieces = tpl["sc_pieces"]
    grp_hi = tpl["grp_hi"]

    nc = bass.Bass()
    featT_h = nc.declare_dram_parameter("featT", [128, NCH, B], BF16, isOutput=False)
    cols_h = nc.declare_dram_parameter("cols", [128, 3 * NCH], F32, isOutput=False)
    w2blk_h = nc.declare_dram_parameter("W2blk", [128, NCH, 128], BF16, isOutput=False)
    spack_h = nc.declare_dram_parameter("Spack", [128, ncols], BF16, isOutput=False)
    b3p_h = nc.declare_dram_parameter("B3p", [1, GSH], BF16, isOutput=False)
    out_h = nc.declare_dram_parameter("out", [B, GSH], BF16, isOutput=True)

    def slot_w(j):
        return min(GSH - 512 * j, 512)

    with ExitStack() as es:
        featT = es.enter_context(nc.sbuf_tensor("ft_sb", [128, NCH, B], BF16))
        colsb = es.enter_context(nc.sbuf_tensor("cols_sb", [128, 3 * NCH], F32))
        w2blk = es.enter_context(nc.sbuf_tensor("w2_sb", [128, NCH, 128], BF16))
        spk = es.enter_context(nc.sbuf_tensor("spk_sb", [128, ncols], BF16))
        b3sb = es.enter_context(nc.sbuf_tensor("b3_sb", [1, GSH], BF16))
        ones = es.enter_context(nc.sbuf_tensor("ones_sb", [1, 128], BF16))
        pre = es.enter_context(nc.sbuf_tensor("pre_sb", [128, 2, B], BF16))
        h1 = es.enter_context(nc.sbuf_tensor("h1_sb", [128, NCH, B], BF16))
        h2 = es.enter_context(nc.sbuf_tensor("h2_sb", [128, NCH, B], BF16))
        outsb = es.enter_context(nc.sbuf_tensor("out_sb", [128, NBT, 512 * NSLOT], BF16))
        pm = [es.enter_context(nc.psum_tensor(f"pm{j}", [128, 512], F32))
              for j in range(8)]

        w1a = colsb[:, 0:NCH]
        b1a = colsb[:, NCH : 2 * NCH]
        b2a = colsb[:, 2 * NCH : 3 * NCH]

        # Scalar-queue DMA chain: cols, fq0, fq1, fq2, fq3 (dsS: 16,32,..,80)
        # Sync-queue DMA chain: w2blk, b3p, sp0..sp7 (dsQ: 16,32,48,..,160)
        with (
            nc.Block() as block,
            nc.semaphore("dsS") as dsS,        # scalar-queue DMA completions
            nc.semaphore("dsQ") as dsQ,        # sync-queue DMA completions
            nc.semaphore("h1s") as sem_h1,     # DVE h1, 1 per chunk
            nc.semaphore("peh") as sem_peh,    # PE w2-mm per chunk
            nc.semaphore("act") as sem_act,    # ACT h2, 1 per chunk
            nc.semaphore("pem") as sem_pem,    # PE bank complete (b3-mm)
            nc.semaphore("ev") as sem_ev,      # DVE evictions (ordered)
            nc.semaphore("od") as sem_od,      # out DMA
        ):
            def ev_wait(engine, m, j):
                """Wait for the previous tenant of bank BANK(m,j) to evict."""
                prev = {(1, 3): (0, 0), (1, 4): (0, 1), (2, 0): (0, 2),
                        (2, 1): (0, 3), (2, 2): (0, 4), (2, 3): (1, 0),
                        (2, 4): (1, 1), (3, 0): (1, 2), (3, 1): (1, 3),
                        (3, 2): (1, 4), (3, 3): (2, 0), (3, 4): (2, 1)}.get((m, j))
                if prev is not None:
                    engine.wait_ge(sem_ev, EV_RANK[prev] + 1)

            @block.scalar
            def _(scalar: bass.BassEngine):
                # issue featT/cols DMAs on the ACT HWDGE ring, then run h2
                scalar.dma_start(out=colsb[:], in_=cols_h[:]).then_inc(dsS, 16)
                for q in range(4):
                    scalar.dma_start(
                        out=featT[:, 4 * q : 4 * (q + 1), :],
                        in_=featT_h[:, 4 * q : 4 * (q + 1), :],
                    ).then_inc(dsS, 16)
                for c in range(NCH):
                    scalar.wait_ge(sem_peh, c + 1)
                    scalar.activation(
                        h2[:, c, :], pm[5 + c % 2][:, :], AFT.Prelu,
                        bias=b2a[:, c : c + 1], alpha=ALPHA,
                    ).then_inc(sem_act)

            @block.sync
            def _(sync: bass.BassEngine):
                sync.dma_start(out=w2blk[:], in_=w2blk_h[:]).then_inc(dsQ, 16)
                sync.dma_start(out=b3sb[:], in_=b3p_h[:]).then_inc(dsQ, 16)
                sp_bounds = [0] + list(grp_hi)
                for q in range(NSC):
                    lo, hi = sp_bounds[q], sp_bounds[q + 1]
                    sync.dma_start(
                        out=spk[:, lo : max(hi, lo + 1)],
                        in_=spack_h[:, lo : max(hi, lo + 1)],
                    ).then_inc(dsQ, 16)
                for e, (m, j) in enumerate(EV_LIST):
                    sync.wait_ge(sem_ev, e + 1)
                    w = slot_w(j)
                    sync.dma_start(
                        out=out_h[m * 128 : (m + 1) * 128, 512 * j : 512 * j + w],
                        in_=outsb[:, m, 512 * j : 512 * j + w],
                    ).then_inc(sem_od, 16)
                sync.wait_ge(sem_od, 16 * len(EV_LIST))

            @block.vector
            def _(vector: bass.BassEngine):
                vector.memset(ones[:], 1.0)
                # h1 = max(x, 0.01x), x = featT*w1 + b1  (2 DVE ops per chunk)
                for c in range(NCH):
                    vector.wait_ge(dsS, 32 + 16 * (c // 4))
                    p = pre[:, c % 2, :]
                    vector.tensor_scalar(
                        p, featT[:, c, :],
                        w1a[:, c : c + 1], b1a[:, c : c + 1],
                        ALU.mult, ALU.add,
                    )
                    vector.scalar_tensor_tensor(
                        h1[:, c, :], p, ALPHA, p, ALU.mult, ALU.max,
                    ).then_inc(sem_h1)
                for e, (m, j) in enumerate(EV_LIST):
                    w = slot_w(j)
                    vector.wait_ge(sem_pem, e + 1)
                    vector.tensor_scalar_add(
                        outsb[:, m, 512 * j : 512 * j + w],
                        pm[BANK(m, j)][:, :w], 0.0,
                    ).then_inc(sem_ev)

            @block.tensor
            def _(tensor: bass.BassEngine):
                def warm(k, n=512):
                    for _ in range(k):
                        tensor.matmul(
                            pm[7][:, :n], featT[:, 0, 0:128], featT[:, 0, :n],
                            start=True, stop=True, skip_group_check=True,
                        )

                def emit_runs(m, sc, slots):
                    for is_start, sub, plo, phi, slo in sc_pieces[sc]:
                        j = plo // 512
                        if j not in slots:
                            continue
                        w = phi - plo
                        tensor.matmul(
                            pm[BANK(m, j)][:, plo - 512 * j : phi - 512 * j],
                            h2[:, SUBS * sc + sub, m * 128 : (m + 1) * 128],
                            spk[:, slo : slo + w],
                            start=is_start, stop=False, skip_group_check=True,
                        )

                def b3mm(m, j):
                    w = slot_w(j)
                    tensor.matmul(
                        pm[BANK(m, j)][:, :w], ones[0:1, 0:128],
                        b3sb[0:1, 512 * j : 512 * j + w],
                        start=False, stop=True, skip_group_check=True,
                    ).then_inc(sem_pem)

                def w2mm(c):
                    if c == 0:
                        tensor.wait_ge(dsQ, 16)         # w2blk
                    tensor.wait_ge(sem_h1, c + 1)       # h1(c) written (DVE)
                    tensor.matmul(
                        pm[5 + c % 2][:, :], w2blk[:, c, :], h1[:, c, :],
                        start=True, stop=True,
                    ).then_inc(sem_peh)

                warm(NWARM)
                # build + btile0 (+ btile1's bank-7 slot j=2)
                for sc in range(NSC):
                    w2mm(2 * sc)
                    w2mm(2 * sc + 1)
                    tensor.wait_ge(sem_act, 2 * sc + 2)  # h2 ready
                    tensor.wait_ge(dsQ, 48 + 16 * sc)    # spack group
                    emit_runs(0, sc, (0, 1, 2, 3, 4))
                    emit_runs(1, sc, (2,))
                tensor.wait_ge(dsQ, 32)                  # b3sb
                for j in range(5):
                    b3mm(0, j)
                b3mm(1, 2)
                # btile1 slots 0,1 (banks 5,6 -- free once ACT consumed ph)
                for sc in range(NSC):
                    emit_runs(1, sc, (0, 1))
                b3mm(1, 0)
                b3mm(1, 1)
                # btile1 slots 3,4 (banks 0,1 <- evictions of t0 j0,j1)
                ev_wait(tensor, 1, 3)
                ev_wait(tensor, 1, 4)
                for sc in range(NSC):
                    emit_runs(1, sc, (3, 4))
                b3mm(1, 3)
                b3mm(1, 4)
                # btile2
                for j in range(5):
                    ev_wait(tensor, 2, j)
                for sc in range(NSC):
                    emit_runs(2, sc, (0, 1, 2, 3, 4))
                for j in range(5):
                    b3mm(2, j)
                # btile3
                for j in range(5):
                    ev_wait(tensor, 3, j)
                for sc in range(NSC):
                    emit_runs(3, sc, (0, 1, 2, 3, 4))
                for j in range(5):
                    b3mm(3, j)

    return nc


def kernel(features, w1, b1, w2, b2, w3, b3, edge_tf):
    global LAST_RESULT
    features, w1, b1, w2, b2, w3, b3, edge_tf = (
        np.asarray(x) for x in (features, w1, b1, w2, b2, w3, b3, edge_tf)
    )
    key = hash(edge_tf.tobytes())
    if key not in _CACHE:
        tpl = _build_template(edge_tf)
        _CACHE.clear()
        _CACHE[key] = (tpl, _build_graph(tpl))
    tpl, graph = _CACHE[key]

    in_maps, gcore = _prep_inputs(
        tpl, features, w1, b1, w2, b2, w3, b3, edge_tf)
    trace = bool(int(os.environ.get("KERNEL_TRACE", "0")))
    if trace:
        _ensure_profile_hook()
    _enable_ldw_opt()
    res = run_bass_kernel_spmd(
        graph, in_maps, core_ids=list(range(NCORES)), trace=trace,
    )
    LAST_RESULT = res
    out = np.zeros((B, G), np.float32)
    for core in range(NCORES):
        dev = np.asarray(res.results[core]["out"]).astype(np.float32)
        out[:, gcore[core]] = dev
    return out


# revision 5
# speedup vs baseline: 1.4298x; 1.2568x over previous
"""Trainium2 Bass kernel for the grouped TF->gene sparse decoder (AEDecoder).

Math (reference):
  h1 = leaky_relu(features[:,:,None] * w1 + b1)            # [B,T,K]
  h2 = leaky_relu(einsum('btj,tjk->btk', h1, w2) + b2)     # [B,T,K]
  out = einsum('bgek,gek->bg', h2[:, edge_tf, :], w3) + b3 # [B,G]

Sparse run-length formulation:
  The final contraction touches only 12 of the 2048 (t,k) rows per gene
  (3 edges x K).  Rows fall in 8 superchunks of 256; a gene touches
  <=3 distinct superchunks (avg 2.64).  Genes are sorted globally by their
  (c1<=c2<=c3) triple and dealt round-robin to the 8 cores, so all
  cores share ONE instruction template while the S data differs per core.
  The host un-permutes the gene order at gather.

Schedule (v2 — HAM-warm + parallel DMA + split h-build):
  * The PE HAM clock gate defaults to 1.2 GHz and only reaches 2.4 GHz
    after ~3.4us of sustained matmul activity.  Warmup matmuls run from
    block entry through the DMA-wait window so the real stream starts and
    stays warm (no PE gaps > ~1us mid-kernel).
  * Input DMA is split across both HWDGE rings: featT+cols issue from the
    scalar (ACT) engine queue, w2blk+b3+spack from the sync queue.  This
    parallelizes the ~0.7us/descriptor issue cost and lands featT ~6us
    earlier.
  * h-build is pipelined across three engines: DVE computes h1
    (tensor_scalar affine + scalar_tensor_tensor leaky max(x, 0.01x)),
    PE does the block-diag w2 matmul (psum ping-pong banks 5,6), ACT does
    h2 = Prelu(psum + b2).  Per-chunk critical path drops ~2.6us -> ~0.75us.
  * Main stream: per batch-tile, runs accumulate into psum banks in
    ascending superchunk order (one start=True per bank); b3 is added by a
    contraction-1 matmul closing each bank; DVE evicts psum -> bf16 SBUF;
    per-bank out DMA on the sync queue.  The 8 psum banks rotate through
    4 btiles x 5 bank-slots; btile1's bank-7 slot runs during the build.

Sharding: 8 cores x 2500 genes (dealt), full batch per core; out bf16
[512, 2500] per core, host casts to fp32 and un-permutes.
"""

import os

import numpy as np
import ml_dtypes

import concourse.bass as bass
import concourse.mybir as mybir
from concourse.bass_utils import run_bass_kernel_spmd

BF16 = mybir.dt.bfloat16
F32 = mybir.dt.float32
AFT = mybir.ActivationFunctionType
ALU = mybir.AluOpType

B, T, K, G, EPG = 512, 512, 4, 20000, 3
NCORES = 8
GSH = G // NCORES            # 2500 genes per core
NCH = (T * K) // 128         # 16 contract chunks (h-build granularity)
NSC = 8                      # 8 superchunks of 256 rows for the main matmul
SUBS = 2                     # partition chunks per superchunk
NBT = B // 128               # 4 batch tiles
NSLOT = (GSH + 511) // 512   # 5 psum bank-slots per btile
ALPHA = 0.01
NWARM = 9                    # HAM warmup matmuls (512 cols, cold ~0.43us each)

# (btile, slot) -> psum bank ring; b3/eviction order = PE completion order
BANK = lambda m, j: (5 * m + j) % 8
EV_LIST = ([(0, j) for j in range(5)] + [(1, 2), (1, 0), (1, 1), (1, 3), (1, 4)]
           + [(2, j) for j in range(5)] + [(3, j) for j in range(5)])
EV_RANK = {mj: e for e, mj in enumerate(EV_LIST)}

_CACHE = {}
LAST_RESULT = None
_LDW_PATCHED = False


def _enable_ldw_opt():
    """Flip walrus --enable-ldw-opt to true: elides redundant LDWEIGHTS for
    back-to-back matmuls sharing a stationary operand (our per-chunk run
    lists reuse one h2 block across ~45 matmuls)."""
    global _LDW_PATCHED
    if _LDW_PATCHED:
        return
    import concourse.bass_utils as bu
    orig = bu.run_command

    def _run(cmd, **kw):
        new = ["--enable-ldw-opt=true" if c == "--enable-ldw-opt=false" else c
               for c in cmd]
        return orig(new, **kw)

    bu.run_command = _run
    _LDW_PATCHED = True


def _ensure_profile_hook():
    """Register an NTFF profile hook when the image lacks antenv.axon_hooks."""
    import contextlib
    import ctypes
    import sys
    import types

    try:
        import antenv.axon_hooks  # noqa: F401
        return
    except ImportError:
        pass

    holder = {}
    mod = types.ModuleType("antenv.axon_hooks")
    mod.set_axon_ntff_profile_hook = lambda h: holder.__setitem__("h", h)
    mod.get_axon_ntff_profile_hook = lambda: holder.get("h")
    sys.modules["antenv.axon_hooks"] = mod

    so_path = "/opt/axon/libaxon_pjrt.so"
    try:
        lib = ctypes.CDLL(so_path)
    except OSError:
        return
    if not hasattr(lib, "axon_start_nrt_profile"):
        return
    lib.axon_start_nrt_profile.argtypes = [
        ctypes.POINTER(ctypes.c_int64), ctypes.c_size_t,
    ]
    lib.axon_start_nrt_profile.restype = ctypes.c_int64
    lib.axon_stop_nrt_profile.argtypes = [ctypes.c_char_p]
    lib.axon_stop_nrt_profile.restype = ctypes.c_int64

    @contextlib.contextmanager
    def _hook(output_dir, device_ids):
        import jax
        jax.devices()
        if device_ids:
            ids = (ctypes.c_int64 * len(device_ids))(*device_ids)
            rc = lib.axon_start_nrt_profile(ids, len(device_ids))
        else:
            rc = lib.axon_start_nrt_profile(None, 0)
        if rc != 0:
            raise RuntimeError(f"axon_start_nrt_profile rc={rc}")
        try:
            yield
        finally:
            n = lib.axon_stop_nrt_profile(str(output_dir).encode())
            print(f"profile: {n} ntff file(s) written to {output_dir}")

    holder["h"] = _hook

    import concourse.bass_utils as bu
    bu.upload_artifacts = lambda tmpdir: tmpdir


# ---------------------------------------------------------------------------
# Template: global gene sort + round-robin deal -> per-chunk piece lists
# shared by all 8 cores.  Pure function of edge_tf.
# ---------------------------------------------------------------------------

def _build_template(edge_tf):
    chunk = edge_tf // 64                      # [G, EPG] superchunk (256 rows)
    keys = np.full((G, 3), NSC, np.int64)      # sorted distinct, pad NSC
    for g in range(G):
        cs = sorted(set(chunk[g].tolist()))
        keys[g, : len(cs)] = cs
    order = np.lexsort((keys[:, 2], keys[:, 1], keys[:, 0]))
    sk = keys[order]

    def blocks(ncols):
        a = sk[:, :ncols]
        change = np.any(a[1:] != a[:-1], axis=1)
        bounds = [0] + (np.nonzero(change)[0] + 1).tolist() + [len(a)]
        for i in range(len(bounds) - 1):
            yield tuple(a[bounds[i]].tolist()), bounds[i], bounds[i + 1]

    # runs: (sc, kind, lo, hi, blockkey, level); positions in [0, GSH)
    runs = []
    l1 = list(blocks(1))
    for i, ((c1,), A, Bb) in enumerate(l1):
        lo, hi = (A + 7) // 8, Bb // 8
        if hi > lo:
            runs.append((c1, "start", lo, hi, (c1,), 1))
        if Bb % 8 != 0 and Bb < G:
            c1n = l1[i + 1][0][0]
            runs.append((c1, "amb_s", Bb // 8, Bb // 8 + 1, (c1,), 1))
            runs.append((c1n, "amb_a", Bb // 8, Bb // 8 + 1, (c1n,), 1))
    for (c1, c2), A, Bb in blocks(2):
        if c2 == NSC:
            continue
        runs.append((c2, "accum", A // 8, (Bb + 7) // 8, (c1, c2), 2))
    for (c1, c2, c3), A, Bb in blocks(3):
        if c3 == NSC:
            continue
        runs.append((c3, "accum", A // 8, (Bb + 7) // 8, (c1, c2, c3), 3))

    # emission order: by superchunk ascending; within one, starts first
    kindord = {"start": 0, "amb_s": 1, "amb_a": 2, "accum": 3}
    runs.sort(key=lambda r: (r[0], kindord[r[1]], r[2]))

    # Each run expands to SUBS matmuls (contraction 256 = 2 partition chunks);
    # spack stores the run's sub-0 block then sub-1 block.  Pieces split at
    # psum bank (512-col) boundaries.
    # HW: start=True resets the ENTIRE psum bank, so exactly one matmul per
    # bank-slot (the first in emission order) carries start=True; everything
    # else accumulates onto the zeroed bank.
    pieces = []          # (sc, psum_lo, psum_hi, spack_lo_run, run_lo, width)
    run_off = []         # spack offset of each run (sub-0 block)
    off = 0
    for c, kind, lo, hi, bk, lvl in runs:
        run_off.append(off)
        p = lo
        while p < hi:
            q = min(hi, (p // 512 + 1) * 512)
            pieces.append((c, p, q, off, lo, hi - lo))
            p = q
        off += SUBS * (hi - lo)
    ncols = off

    # sc_pieces[S] = [(is_start, sub, plo, phi, slo), ...] emission order:
    # sub-major within a superchunk so same-stationary matmuls are adjacent
    sc_pieces = {c: [] for c in range(NSC)}
    tmp = {c: [] for c in range(NSC)}
    for c, plo, phi, off0, rlo, rw in pieces:
        tmp[c].append((plo, phi, off0, rlo, rw))
    slot_seen = set()
    for c in range(NSC):
        for sub in range(SUBS):
            for plo, phi, off0, rlo, rw in tmp[c]:
                slo = off0 + sub * rw + (plo - rlo)
                j = plo // 512
                is_start = j not in slot_seen
                slot_seen.add(j)
                sc_pieces[c].append((is_start, sub, plo, phi, slo))
    # spack DMA groups: one per superchunk
    grp_hi = []
    for jc in range(NSC):
        nxt = [run_off[i] for i, r in enumerate(runs) if r[0] > jc]
        grp_hi.append(min(nxt) if nxt else ncols)

    return dict(keys=keys, order=order, runs=runs, run_off=run_off,
                ncols=ncols, sc_pieces=sc_pieces, grp_hi=grp_hi,
                chunkmap=chunk)


# ---------------------------------------------------------------------------
# Host data packing (layout/index preprocessing only)
# ---------------------------------------------------------------------------

def _prep_inputs(tpl, features, w1, b1, w2, b2, w3, b3, edge_tf):
    bf = ml_dtypes.bfloat16
    keys, order, runs = tpl["keys"], tpl["order"], tpl["runs"]
    run_off, ncols = tpl["run_off"], tpl["ncols"]

    featT = np.repeat(np.ascontiguousarray(features.T), K, axis=0)
    featT = np.ascontiguousarray(
        featT.reshape(NCH, 128, B).transpose(1, 0, 2)).astype(bf)

    w1c = w1.reshape(T * K).reshape(NCH, 128).T.astype(np.float32)
    b1c = b1.reshape(T * K).reshape(NCH, 128).T.astype(np.float32)
    b2c = b2.reshape(T * K).reshape(NCH, 128).T.astype(np.float32)
    cols = np.concatenate([w1c, b1c, b2c], axis=1).copy()

    w2r = w2.reshape(NCH, 32, K, K)
    w2blk = np.zeros((NCH, 32, K, 32, K), np.float32)
    for i in range(32):
        w2blk[:, i, :, i, :] = w2r[:, i]
    w2blk = np.ascontiguousarray(
        w2blk.reshape(NCH, 128, 128).transpose(1, 0, 2)).astype(bf)

    # per-gene merged columns per distinct superchunk slot, per sub-chunk
    gcol = np.zeros((G, 3, SUBS, 128), np.float32)
    gidx = np.arange(G)
    for e in range(EPG):
        t = edge_tf[:, e]
        cc = t // 64
        s = np.argmax(keys == cc[:, None], axis=1)
        sub = (t % 64) // 32
        rows = 4 * (t % 32)
        for k in range(K):
            np.add.at(gcol, (gidx, s, sub, rows + k), w3[:, e, k])

    gcore = np.empty((NCORES, GSH), np.int64)      # position -> original gene
    for core in range(NCORES):
        gcore[core] = order[np.arange(GSH) * 8 + core]

    spack = np.zeros((NCORES, 128, ncols), np.float32)
    for ri, (c, kind, lo, hi, bk, lvl) in enumerate(runs):
        w = hi - lo
        o = run_off[ri]
        ps = np.arange(lo, hi)
        for core in range(NCORES):
            genes = gcore[core][ps]
            kk = keys[genes]
            member = kk[:, 0] == bk[0]
            for d in range(1, lvl):
                member &= kk[:, d] == bk[d]
            s = np.argmax(kk == c, axis=1)
            for sub in range(SUBS):
                vals = np.where(member[:, None], gcol[genes, s, sub, :], 0.0)
                spack[core, :, o + sub * w : o + (sub + 1) * w] = vals.T
    spack = spack.astype(bf)

    b3p = np.zeros((NCORES, 1, GSH), np.float32)
    for core in range(NCORES):
        b3p[core, 0, :] = b3[gcore[core]]
    b3p = b3p.astype(bf)

    in_maps = []
    for core in range(NCORES):
        in_maps.append({
            "featT": featT,
            "cols": cols,
            "W2blk": w2blk,
            "Spack": np.ascontiguousarray(spack[core]),
            "B3p": np.ascontiguousarray(b3p[core]),
        })
    return in_maps, gcore


# ---------------------------------------------------------------------------
# Graph
# ---------------------------------------------------------------------------

def _build_graph(tpl):
    from contextlib import ExitStack

    ncols = tpl["ncols"]
    sc_pieces = tpl["sc_pieces"]
    grp_hi = tpl["grp_hi"]

    nc = bass.Bass()
    featT_h = nc.declare_dram_parameter("featT", [128, NCH, B], BF16, isOutput=False)
    cols_h = nc.declare_dram_parameter("cols", [128, 3 * NCH], F32, isOutput=False)
    w2blk_h = nc.declare_dram_parameter("W2blk", [128, NCH, 128], BF16, isOutput=False)
    spack_h = nc.declare_dram_parameter("Spack", [128, ncols], BF16, isOutput=False)
    b3p_h = nc.declare_dram_parameter("B3p", [1, GSH], BF16, isOutput=False)
    out_h = nc.declare_dram_parameter("out", [B, GSH], BF16, isOutput=True)

    def slot_w(j):
        return min(GSH - 512 * j, 512)

    with ExitStack() as es:
        featT = es.enter_context(nc.sbuf_tensor("ft_sb", [128, NCH, B], BF16))
        colsb = es.enter_context(nc.sbuf_tensor("cols_sb", [128, 3 * NCH], F32))
        w2blk = es.enter_context(nc.sbuf_tensor("w2_sb", [128, NCH, 128], BF16))
        spk = es.enter_context(nc.sbuf_tensor("spk_sb", [128, ncols], BF16))
        b3sb = es.enter_context(nc.sbuf_tensor("b3_sb", [1, GSH], BF16))
        ones = es.enter_context(nc.sbuf_tensor("ones_sb", [1, 128], BF16))
        pre = es.enter_context(nc.sbuf_tensor("pre_sb", [128, 2, B], BF16))
        h1 = es.enter_context(nc.sbuf_tensor("h1_sb", [128, NCH, B], BF16))
        h2 = es.enter_context(nc.sbuf_tensor("h2_sb", [128, NCH, B], BF16))
        outsb = es.enter_context(nc.sbuf_tensor("out_sb", [128, NBT, 512 * NSLOT], BF16))
        pm = [es.enter_context(nc.psum_tensor(f"pm{j}", [128, 512], F32))
              for j in range(8)]

        w1a = colsb[:, 0:NCH]
        b1a = colsb[:, NCH : 2 * NCH]
        b2a = colsb[:, 2 * NCH : 3 * NCH]

        # Single sync-ring input chain (SDMA completion sems can fire a hair
        # before the slowest engine's data is visible in SBUF, so every
        # consumer gate waits through the NEXT transfer's completion --
        # one-transfer slack; a dummy tail transfer backs the last gates):
        #   cols, fq0, w2blk, sp0, fq1, sp1, fq2, sp2, fq3,
        #   sp3, sp4, sp5, sp6, sp7, b3p, tail(w2blk again)
        CHAIN = ["cols", "fq0", "w2blk", "sp0", "fq1", "sp1", "fq2", "sp2",
                 "fq3", "sp3", "sp4", "sp5", "sp6", "sp7", "b3p", "tail"]
        POS = {name: 16 * (i + 1) for i, name in enumerate(CHAIN)}

        def gate(name):
            """dsync value that guarantees `name` landed (next transfer done)."""
            i = CHAIN.index(name)
            return POS[CHAIN[min(i + 1, len(CHAIN) - 1)]]

        with (
            nc.Block() as block,
            nc.semaphore("dsync") as dsync,    # input DMA chain completions
            nc.semaphore("h1s") as sem_h1,     # DVE h1, 1 per chunk
            nc.semaphore("peh") as sem_peh,    # PE w2-mm per chunk
            nc.semaphore("act") as sem_act,    # ACT h2, 1 per chunk
            nc.semaphore("pem") as sem_pem,    # PE bank complete (b3-mm)
            nc.semaphore("ev") as sem_ev,      # DVE evictions (ordered)
            nc.semaphore("od") as sem_od,      # out DMA
        ):
            def ev_wait(engine, m, j):
                """Wait for the previous tenant of bank BANK(m,j) to evict."""
                prev = {(1, 3): (0, 0), (1, 4): (0, 1), (2, 0): (0, 2),
                        (2, 1): (0, 3), (2, 2): (0, 4), (2, 3): (1, 0),
                        (2, 4): (1, 1), (3, 0): (1, 2), (3, 1): (1, 3),
                        (3, 2): (1, 4), (3, 3): (2, 0), (3, 4): (2, 1)}.get((m, j))
                if prev is not None:
                    engine.wait_ge(sem_ev, EV_RANK[prev] + 1)

            @block.scalar
            def _(scalar: bass.BassEngine):
                for c in range(NCH):
                    scalar.wait_ge(sem_peh, c + 1)
                    scalar.activation(
                        h2[:, c, :], pm[5 + c % 2][:, :], AFT.Prelu,
                        bias=b2a[:, c : c + 1], alpha=ALPHA,
                    ).then_inc(sem_act)

            @block.sync
            def _(sync: bass.BassEngine):
                sp_bounds = [0] + list(grp_hi)

                def sp_slice(k):
                    lo, hi = sp_bounds[k], sp_bounds[k + 1]
                    return slice(lo, max(hi, lo + 1))

                xfers = {
                    "cols": (colsb[:], cols_h[:]),
                    "w2blk": (w2blk[:], w2blk_h[:]),
                    "b3p": (b3sb[:], b3p_h[:]),
                    "tail": (w2blk[:], w2blk_h[:]),
                }
                for q in range(4):
                    xfers[f"fq{q}"] = (featT[:, 4 * q : 4 * (q + 1), :],
                                       featT_h[:, 4 * q : 4 * (q + 1), :])
                for k in range(NSC):
                    xfers[f"sp{k}"] = (spk[:, sp_slice(k)],
                                       spack_h[:, sp_slice(k)])
                for name in CHAIN:
                    dst, src = xfers[name]
                    sync.dma_start(out=dst, in_=src).then_inc(dsync, 16)
                for e, (m, j) in enumerate(EV_LIST):
                    sync.wait_ge(sem_ev, e + 1)
                    w = slot_w(j)
                    sync.dma_start(
                        out=out_h[m * 128 : (m + 1) * 128, 512 * j : 512 * j + w],
                        in_=outsb[:, m, 512 * j : 512 * j + w],
                    ).then_inc(sem_od, 16)
                sync.wait_ge(sem_od, 16 * len(EV_LIST))

            @block.vector
            def _(vector: bass.BassEngine):
                vector.memset(ones[:], 1.0)
                # h1 = max(x, 0.01x), x = featT*w1 + b1  (2 DVE ops per chunk)
                for c in range(NCH):
                    vector.wait_ge(dsync, gate(f"fq{c // 4}"))
                    p = pre[:, c % 2, :]
                    vector.tensor_scalar(
                        p, featT[:, c, :],
                        w1a[:, c : c + 1], b1a[:, c : c + 1],
                        ALU.mult, ALU.add,
                    )
                    vector.scalar_tensor_tensor(
                        h1[:, c, :], p, ALPHA, p, ALU.mult, ALU.max,
                    ).then_inc(sem_h1)
                for e, (m, j) in enumerate(EV_LIST):
                    w = slot_w(j)
                    vector.wait_ge(sem_pem, e + 1)
                    vector.tensor_scalar_add(
                        outsb[:, m, 512 * j : 512 * j + w],
                        pm[BANK(m, j)][:, :w], 0.0,
                    ).then_inc(sem_ev)

            @block.tensor
            def _(tensor: bass.BassEngine):
                def warm(k, n=512):
                    for _ in range(k):
                        tensor.matmul(
                            pm[7][:, :n], featT[:, 0, 0:128], featT[:, 0, :n],
                            start=True, stop=True, skip_group_check=True,
                        )

                def emit_runs(m, sc, slots):
                    for is_start, sub, plo, phi, slo in sc_pieces[sc]:
                        j = plo // 512
                        if j not in slots:
                            continue
                        w = phi - plo
                        tensor.matmul(
                            pm[BANK(m, j)][:, plo - 512 * j : phi - 512 * j],
                            h2[:, SUBS * sc + sub, m * 128 : (m + 1) * 128],
                            spk[:, slo : slo + w],
                            start=is_start, stop=False, skip_group_check=True,
                        )

                def b3mm(m, j):
                    w = slot_w(j)
                    tensor.matmul(
                        pm[BANK(m, j)][:, :w], ones[0:1, 0:128],
                        b3sb[0:1, 512 * j : 512 * j + w],
                        start=False, stop=True, skip_group_check=True,
                    ).then_inc(sem_pem)

                def w2mm(c):
                    if c == 0:
                        tensor.wait_ge(dsync, gate("w2blk"))
                    if c >= 2:
                        tensor.wait_ge(sem_act, c - 1)  # bank tenant consumed
                    tensor.wait_ge(sem_h1, c + 1)       # h1(c) written (DVE)
                    tensor.matmul(
                        pm[5 + c % 2][:, :], w2blk[:, c, :], h1[:, c, :],
                        start=True, stop=True,
                    ).then_inc(sem_peh)

                warm(NWARM)
                # build + btile0 (+ btile1's bank-7 slot j=2); w2mm pairs are
                # pre-issued 2 superchunks ahead so ACT h2 overlaps emission
                w2mm(0); w2mm(1); w2mm(2); w2mm(3)
                for sc in range(NSC):
                    if sc < NSC - 2:
                        w2mm(2 * sc + 4)
                        w2mm(2 * sc + 5)
                    tensor.wait_ge(sem_act, 2 * sc + 2)  # h2 ready
                    tensor.wait_ge(dsync, gate(f"sp{sc}"))
                    emit_runs(0, sc, (0, 1, 2, 3, 4))
                    emit_runs(1, sc, (2,))
                tensor.wait_ge(dsync, gate("b3p"))
                for j in range(5):
                    b3mm(0, j)
                b3mm(1, 2)
                # btile1 slots 0,1 (banks 5,6 -- free once ACT consumed ph)
                for sc in range(NSC):
                    emit_runs(1, sc, (0, 1))
                b3mm(1, 0)
                b3mm(1, 1)
                # btile1 slots 3,4 (banks 0,1 <- evictions of t0 j0,j1)
                ev_wait(tensor, 1, 3)
                ev_wait(tensor, 1, 4)
                for sc in range(NSC):
                    emit_runs(1, sc, (3, 4))
                b3mm(1, 3)
                b3mm(1, 4)
                # btile2
                for j in range(5):
                    ev_wait(tensor, 2, j)
                for sc in range(NSC):
                    emit_runs(2, sc, (0, 1, 2, 3, 4))
                for j in range(5):
                    b3mm(2, j)
                # btile3
                for j in range(5):
                    ev_wait(tensor, 3, j)
                for sc in range(NSC):
                    emit_runs(3, sc, (0, 1, 2, 3, 4))
                for j in range(5):
                    b3mm(3, j)

    return nc


def kernel(features, w1, b1, w2, b2, w3, b3, edge_tf):
    global LAST_RESULT
    features, w1, b1, w2, b2, w3, b3, edge_tf = (
        np.asarray(x) for x in (features, w1, b1, w2, b2, w3, b3, edge_tf)
    )
    key = hash(edge_tf.tobytes())
    if key not in _CACHE:
        tpl = _build_template(edge_tf)
        _CACHE.clear()
        _CACHE[key] = (tpl, _build_graph(tpl))
    tpl, graph = _CACHE[key]

    in_maps, gcore = _prep_inputs(
        tpl, features, w1, b1, w2, b2, w3, b3, edge_tf)
    trace = bool(int(os.environ.get("KERNEL_TRACE", "0")))
    if trace:
        _ensure_profile_hook()
    _enable_ldw_opt()
    res = run_bass_kernel_spmd(
        graph, in_maps, core_ids=list(range(NCORES)), trace=trace,
    )
    LAST_RESULT = res
    out = np.zeros((B, G), np.float32)
    for core in range(NCORES):
        dev = np.asarray(res.results[core]["out"]).astype(np.float32)
        out[:, gcore[core]] = dev
    return out
